# revision 1
# baseline (speedup 1.0000x reference)
"""Trainium2 Bass kernel for nn_CrossAttention (B=2, S=C=4096, D=512, H=8, Dh=64).

Sharding: batch x head-pair parallel over 8 cores. Core c handles batch
b = c//4 and heads {2*(c%4), 2*(c%4)+1}. Each core computes full attention
for its two heads plus its partial contribution to the output projection;
the host sums the 4 per-core partials per batch and adds the bias.

Device-side dataflow per core (all transposed layouts, no on-chip
transposes needed):
  qT [128=2*dh, S]  = wqT_slice.T @ xT          (f32r matmuls)
  kT [128=2*dh, C]  = wkT_slice.T @ ctxT
  v  [c, 2*dh]      = ctxT.T @ wvT_slice        -> v_aug [c, 65] with ones col
  sT chunk [128c, 512q] = kT_h_chunk.T @ qT_h   (two heads row-tiled on PE)
  P = exp(SCALE * sT)                            (ACT, f32r out)
  o_aug [65, 512q] += v_aug_chunk.T @ P_chunk    (ones col -> row 64 = denom)
  oT = o_aug[0:64] * (1/denom broadcast via K=1 ones matmul)
  y_partial [128s, 512] = sum_h oT_h_chunk.T @ woT_h

Numerics: f32r (tf32) matmuls with host-side pre-rounding of DRAM inputs;
products of tf32 values accumulate exactly in fp32, so the only error is
the tf32 input rounding (~5e-4) plus exp(2 ULP) and the softmax reciprocal
(~51 ULP from reciprocal_approx_fast).
"""

import os
import numpy as np
from contextlib import ExitStack

import concourse.bass as bass
import concourse.tile as tile
from concourse import bacc, mybir
from concourse.bass_utils import run_bass_kernel_spmd

F32 = mybir.dt.float32
F32R = mybir.dt.float32r
EXP = mybir.ActivationFunctionType.Exp

B = 2
S = 4096
C = 4096
D = 512
DH = 64
SCALE = DH ** -0.5  # 0.125

NQB = S // 512   # 8 query blocks of 512
NCB = C // 128   # 32 context chunks of 128
NKC = D // 128   # 4 contraction chunks of 128
NNC = S // 512   # 8 free-dim chunks of 512 for q/k projections
VW = DH + 1      # 65: v_aug chunk width (ones column at 64)

_CACHE = {}


def round_tf32(a: np.ndarray) -> np.ndarray:
    b = np.ascontiguousarray(a, dtype=np.float32).view(np.uint32)
    b = (b + np.uint32(0x1000)) & np.uint32(0xFFFFE000)
    return b.view(np.float32)


def build_nc():
    nc = bacc.Bacc("TRN2", target_bir_lowering=False, debug=False)
    nqb = int(os.environ.get("ATT_QB", NQB))
    rowtile = os.environ.get("ROWTILE", "1") == "1"

    xT = nc.dram_tensor("xT", [D, S], F32R, kind="ExternalInput").ap()
    ctxT = nc.dram_tensor("ctxT", [D, C], F32R, kind="ExternalInput").ap()
    wqT = nc.dram_tensor("wqT", [D, 128], F32R, kind="ExternalInput").ap()
    wkT = nc.dram_tensor("wkT", [D, 128], F32R, kind="ExternalInput").ap()
    wvT = nc.dram_tensor("wvT", [D, 128], F32R, kind="ExternalInput").ap()
    woT = nc.dram_tensor("woT", [128, D], F32R, kind="ExternalInput").ap()
    vones = nc.dram_tensor("vones", [128, NCB], F32R, kind="ExternalInput").ap()
    onesk = nc.dram_tensor("onesk", [1, DH], F32, kind="ExternalInput").ap()
    y = nc.dram_tensor("y", [S, D], F32, kind="ExternalOutput").ap()
    dbg_den = nc.dram_tensor("dbg_den", [1, 512], F32, kind="ExternalOutput").ap()
    dbg_rc = nc.dram_tensor("dbg_rc", [1, 512], F32, kind="ExternalOutput").ap()

    with tile.TileContext(nc) as tc, ExitStack() as ctx:
        sb = ctx.enter_context(tc.tile_pool(name="sb", bufs=1))

        # ---- persistent SBUF tiles ----
        wq_sb = sb.tile([128, D], F32R, name="wq_sb")
        wk_sb = sb.tile([128, D], F32R, name="wk_sb")
        wv_sb = sb.tile([128, D], F32R, name="wv_sb")
        wo0_sb = sb.tile([64, D], F32R, name="wo0_sb")
        wo1_sb = sb.tile([64, D], F32R, name="wo1_sb")
        onesk_sb = sb.tile([1, DH], F32, name="onesk_sb")
        kT_sb = sb.tile([128, C], F32R, name="kT_sb")
        qT_sb = sb.tile([128, S], F32R, name="qT_sb")
        v0_sb = sb.tile([128, NCB * VW], F32R, name="v0_sb")
        v1_sb = sb.tile([128, NCB * VW], F32R, name="v1_sb")

        for kc in range(NKC):
            nc.sync.dma_start(wq_sb[:, kc * 128:(kc + 1) * 128],
                              wqT[kc * 128:(kc + 1) * 128, :])
            nc.sync.dma_start(wk_sb[:, kc * 128:(kc + 1) * 128],
                              wkT[kc * 128:(kc + 1) * 128, :])
            nc.sync.dma_start(wv_sb[:, kc * 128:(kc + 1) * 128],
                              wvT[kc * 128:(kc + 1) * 128, :])
        nc.sync.dma_start(wo0_sb[:], woT[0:64, :])
        nc.sync.dma_start(wo1_sb[:], woT[64:128, :])
        nc.sync.dma_start(onesk_sb[:], onesk)
        # ones columns of v_aug (position 64 of each 65-wide chunk)
        v0_3d = v0_sb.rearrange("p (c k) -> p c k", k=VW)
        v1_3d = v1_sb.rearrange("p (c k) -> p c k", k=VW)
        nc.sync.dma_start(v0_3d[:, :, 64:65], vones.unsqueeze(2))
        nc.sync.dma_start(v1_3d[:, :, 64:65], vones.unsqueeze(2))

        # ---- one shared PSUM pool; proj borrows the bufs=1 slots ----
        with tc.tile_pool(name="aps", bufs=1, space="PSUM") as aps, \
             tc.tile_pool(name="inbig", bufs=10) as inbig, \
             tc.tile_pool(name="psb", bufs=4) as psb, \
             tc.tile_pool(name="msb", bufs=2) as msb:
            # input halves, attention-critical DMAs first
            ctx_ch = [[None] * 2 for _ in range(NKC)]
            x_ch = [[None] * 2 for _ in range(NKC)]
            for h, arr, src_ap, nm in ((0, ctx_ch, ctxT, "ctx"), (0, x_ch, xT, "x"),
                                       (1, ctx_ch, ctxT, "ctx"), (1, x_ch, xT, "x")):
                for kc in range(NKC):
                    t = inbig.tile([128, 2048], F32R, name=f"{nm}{kc}_{h}",
                                   tag="in")
                    nc.sync.dma_start(t[:], src_ap[kc * 128:(kc + 1) * 128,
                                                   h * 2048:(h + 1) * 2048])
                    arr[kc][h] = t

            def kproj(n):
                h = n // 4
                pk = aps.tile([128, 512], F32, name=f"pk{n}", tag="py", bufs=1)
                for kc in range(NKC):
                    nc.tensor.matmul(pk[:], wk_sb[:, kc * 128:(kc + 1) * 128],
                                     ctx_ch[kc][h][:, (n - 4 * h) * 512:
                                                   (n - 4 * h + 1) * 512],
                                     start=(kc == 0), stop=(kc == NKC - 1))
                nc.vector.tensor_copy(kT_sb[:, n * 512:(n + 1) * 512], pk[:])

            def qproj(n):
                h = n // 4
                pq = aps.tile([128, 512], F32, name=f"pq{n}", tag="py", bufs=1)
                for kc in range(NKC):
                    nc.tensor.matmul(pq[:], wq_sb[:, kc * 128:(kc + 1) * 128],
                                     x_ch[kc][h][:, (n - 4 * h) * 512:
                                                 (n - 4 * h + 1) * 512],
                                     start=(kc == 0), stop=(kc == NKC - 1))
                nc.vector.tensor_copy(qT_sb[:, n * 512:(n + 1) * 512], pq[:])

            def vproj(cb):
                h = cb // 16
                pv = aps.tile([128, 128], F32, name=f"pv{cb}", tag="bc", bufs=1)
                for kc in range(NKC):
                    nc.tensor.matmul(pv[:],
                                     ctx_ch[kc][h][:, (cb - 16 * h) * 128:
                                                   (cb - 16 * h + 1) * 128],
                                     wv_sb[:, kc * 128:(kc + 1) * 128],
                                     start=(kc == 0), stop=(kc == NKC - 1))
                nc.vector.tensor_copy(v0_sb[:, cb * VW:cb * VW + DH], pv[:, 0:64])
                nc.vector.tensor_copy(v1_sb[:, cb * VW:cb * VW + DH], pv[:, 64:128])

            for n in range(4):
                kproj(n)
            qproj(0)

            def pre_work(qb, g):
                # software-pipelined remainder of the projections inside qb0
                if qb == 0:
                    if g == 0:
                        for cb in range(6):
                            vproj(cb)
                    elif g <= 13:
                        vproj(2 * g + 4)
                        vproj(2 * g + 5)
                    if 3 <= g <= 6:
                        kproj(g + 1)
                if g == 0 and qb + 1 < NQB:
                    qproj(qb + 1)

            # ---- attention + output projection ----
            for qb in range(nqb):
                qsl = slice(qb * 512, (qb + 1) * 512)
                po0 = aps.tile([VW, 512], F32, name=f"po0_{qb}", tag="o", bufs=2)
                po1 = aps.tile([VW, 512], F32, name=f"po1_{qb}", tag="o", bufs=2)
                for g in range(NCB // 2):
                    pre_work(qb, g)
                    cb0, cb1 = 2 * g, 2 * g + 1
                    s0 = aps.tile([128, 1024], F32, name=f"s0_{qb}_{g}",
                                  tag="s", bufs=2)
                    s1 = aps.tile([128, 1024], F32, name=f"s1_{qb}_{g}",
                                  tag="s", bufs=2)
                    for i, cb in ((0, cb0), (1, cb1)):
                        csl = slice(cb * 128, (cb + 1) * 128)
                        nc.tensor.matmul(s0[:, i * 512:(i + 1) * 512],
                                         kT_sb[0:64, csl], qT_sb[0:64, qsl],
                                         start=True, stop=True,
                                         tile_position=(0, 0) if rowtile else None)
                        nc.tensor.matmul(s1[:, i * 512:(i + 1) * 512],
                                         kT_sb[64:128, csl], qT_sb[64:128, qsl],
                                         start=True, stop=True,
                                         tile_position=(64, 0) if rowtile else None)
                    p0 = psb.tile([128, 1024], F32R, name=f"p0_{qb}_{g}", tag="p", bufs=6)
                    p1 = psb.tile([128, 1024], F32R, name=f"p1_{qb}_{g}", tag="p", bufs=6)
                    nc.scalar.activation(p0[:], s0[:], EXP, scale=SCALE)
                    nc.scalar.activation(p1[:], s1[:], EXP, scale=SCALE)
                    for i, cb in ((0, cb0), (1, cb1)):
                        vsl = slice(cb * VW, cb * VW + VW)
                        nc.tensor.matmul(po0[:], v0_sb[:, vsl],
                                         p0[:, i * 512:(i + 1) * 512],
                                         start=(g == 0 and i == 0),
                                         stop=(g == NCB // 2 - 1 and i == 1))
                        nc.tensor.matmul(po1[:], v1_sb[:, vsl],
                                         p1[:, i * 512:(i + 1) * 512],
                                         start=(g == 0 and i == 0),
                                         stop=(g == NCB // 2 - 1 and i == 1))
                # softmax normalization: oT = o_aug[0:64] / denom
                ot0 = psb.tile([64, 512], F32R, name=f"ot0_{qb}", tag="ot", bufs=4)
                ot1 = psb.tile([64, 512], F32R, name=f"ot1_{qb}", tag="ot", bufs=4)
                for hl, po, oT in ((0, po0, ot0), (1, po1, ot1)):
                    den = msb.tile([1, 512], F32, name=f"den{hl}_{qb}", tag="den")
                    nc.vector.tensor_copy(den[:], po[64:65, :])
                    rc = msb.tile([1, 512], F32, name=f"rc{hl}_{qb}", tag="rc")
                    nc.vector.reciprocal(rc[:], den[:])
                    if qb == 0 and hl == 0:
                        nc.sync.dma_start(dbg_den, den[:])
                        nc.sync.dma_start(dbg_rc, rc[:])
                    bc = aps.tile([64, 512], F32, name=f"bc{hl}_{qb}",
                                  tag="bc", bufs=1)
                    nc.tensor.matmul(bc[:], onesk_sb[:], rc[:],
                                     start=True, stop=True)
                    bcs = msb.tile([64, 512], F32, name=f"bcs{hl}_{qb}", tag="bcs")
                    nc.vector.tensor_copy(bcs[:], bc[:])
                    nc.vector.tensor_mul(oT[:], po[0:64, :], bcs[:])
                # output projection for this q-block
                for sc in range(4):
                    r0 = qb * 512 + sc * 128
                    ssl = slice(r0, r0 + 128)
                    py = aps.tile([128, D], F32, name=f"py_{qb}_{sc}",
                                  tag="py", bufs=1)
                    nc.tensor.matmul(py[:], ot0[:, sc * 128:(sc + 1) * 128],
                                     wo0_sb[:], start=True, stop=False)
                    nc.tensor.matmul(py[:], ot1[:, sc * 128:(sc + 1) * 128],
                                     wo1_sb[:], start=False, stop=True)
                    ysb = msb.tile([128, D], F32, name=f"y_{qb}_{sc}", tag="y")
                    nc.vector.tensor_copy(ysb[:], py[:])
                    nc.sync.dma_start(y[ssl, :], ysb[:])

    nc.compile()
    return nc


def make_in_maps(x, context, w_q, w_k, w_v, w_out):
    wqT = round_tf32(w_q.T)    # [D, INNER]
    wkT = round_tf32(w_k.T)
    wvT = round_tf32(w_v.T)
    woT = round_tf32(w_out.T)  # [INNER, D]
    vones = np.ones((128, NCB), dtype=np.float32)
    onesk = np.ones((1, DH), dtype=np.float32)
    xTs = [round_tf32(x[b].T) for b in range(B)]
    cTs = [round_tf32(context[b].T) for b in range(B)]
    in_maps = []
    for c in range(8):
        b, hp = c // 4, c % 4
        hsl = slice(hp * 128, (hp + 1) * 128)
        in_maps.append({
            "xT": xTs[b],
            "ctxT": cTs[b],
            "wqT": np.ascontiguousarray(wqT[:, hsl]),
            "wkT": np.ascontiguousarray(wkT[:, hsl]),
            "wvT": np.ascontiguousarray(wvT[:, hsl]),
            "woT": np.ascontiguousarray(woT[hsl, :]),
            "vones": vones,
            "onesk": onesk,
        })
    return in_maps


def kernel(x, context, w_q, w_k, w_v, w_out, b_out):
    x = np.asarray(x, dtype=np.float32)
    context = np.asarray(context, dtype=np.float32)
    w_q = np.asarray(w_q, dtype=np.float32)
    w_k = np.asarray(w_k, dtype=np.float32)
    w_v = np.asarray(w_v, dtype=np.float32)
    w_out = np.asarray(w_out, dtype=np.float32)
    b_out = np.asarray(b_out, dtype=np.float32)

    if "nc" not in _CACHE:
        _CACHE["nc"] = build_nc()
    nc = _CACHE["nc"]

    in_maps = make_in_maps(x, context, w_q, w_k, w_v, w_out)
    res = run_bass_kernel_spmd(nc, in_maps, list(range(8))).results
    _CACHE["res0"] = res[0]

    out = np.zeros((B, S, D), dtype=np.float32)
    for c in range(8):
        out[c // 4] += res[c]["y"]
    out += b_out
    return out

